# revision 19
# baseline (speedup 1.0000x reference)
"""v9: fp16 transposed layout; plain-DMA kernel streaming from a
host-pretransposed [j, i, k''] tensor (1.6x faster than the old DMA-xbar
transpose path and far lighter on the sync sequencer); per-row work is
split 50/50 between engines with fused DVE ops (STT_MODE): on each row,
the ACT-assigned channels (2 on even row-pairs, 1 on odd -- 192 of 384
total) get their products from a DVE tensor_tensor (2x fp16) restricted
to just those channels and are reduced by ScalarE activation-accum; the
DVE-assigned channels skip the separate product pass entirely via
scalar_tensor_tensor (multiply + accumulate in one 1x pass). Interleaved
repeat-slope benchmarking on the real cores measured this 36% faster
than the unfused TT+tensor_scalar structure (137.8us vs 214.6us slope),
with a sharp optimum at the 50/50 alternating share. Reduces are pure
sums; the 1/361 scale is folded into the final PSUM->SBUF copy after
the PE transpose. gpsimd is unused: its tensor_scalar/STT accum crashes
the runtime, and its tensor_tensor multiply helped in some device
sessions but regressed badly in others.

The sliding x-windows (even and odd-row variants) are materialized
VERBATIM on the host and each lands in ONE flat contiguous DMA
(16.9KB/partition linear descriptors, WIN_MERGE) instead of 18 small
strided window-build DMAs: -16us measured on silicon from the lighter
sequencer/descriptor load, at a ~+5us cost in first-row ramp.

Per core: partitions = 128 output cols (j). Kernel tap axis k'' = v*20+u
(19 v-rows of 20 slots, u=19 slot zero; rows 380..383 zero) so every
innermost run is 20 elements and 4-byte aligned -- the conditions for
DVE 2x fp16 perf mode on real silicon.

x: transposed + fp16 on host; device holds TWO sliding col-windows, the
second shifted one row, so the fused multiply's innermost run start is
always even (silicon 2x alignment).
"""

import numpy as np

import concourse.bacc as bacc
import concourse.mybir as mybir
import concourse.tile as tile
from concourse import bass_utils
from concourse.ap import AP

L = 19
K2 = L * L
VS = 20            # padded v-row stride (taps per v-row incl. zero slot)
NT = L * VS        # 380 product slots per channel
K2P = 384          # padded tap rows (3 x 128)
PAD = L // 2
B, C, H, W = 2, 3, 256, 256
BLK = 128
XS = BLK + L - 1   # 146 valid cols
XSP = 148          # padded row stride (even)
IB = 16            # i-rows per kernel-stream block

_CACHE = {}
LAST_EXEC_NS = None

# --- tuning knobs -----------------------------------------------------
CFG = "v8"             # v8 | dmaonly | noreduce (ablations)
XWO_DMA = True         # build odd-row window by DMA instead of gpsimd
POOL_ROWS = ()         # i%8 values whose multiply runs on Pool engine
TR_ROWS = ()           # i%8 values whose reduce is one DVE tensor_reduce
SPLIT_W = (1, 1)       # (dve_ts, act) weights for remaining reduces
STT_MODE = True        # fused scalar_tensor_tensor for DVE channels; TT
                       # covers only the ACT-assigned channels
ACT2_OF4 = 2           # rows per 4 whose ACT share is 2 channels (vs 1)
ACT2_ALT = False       # strict per-row alternation of the 2/1 ACT share
PR_BUFS = 8            # product/scratch ring depth
WIN_MERGE = True       # build each window half with one 4D-AP DMA
DMA_LEAN = False       # 32-row blocks + merged out DMA: regressed on hw


def _split_engines(n, w):
    """Deterministic weighted round-robin over len(w) engines."""
    credits = [0.0] * len(w)
    tot = float(sum(w))
    out = []
    for _ in range(n):
        for e in range(len(w)):
            credits[e] += w[e] / tot
        e = max(range(len(w)), key=lambda x: credits[x])
        credits[e] -= 1.0
        out.append(e)
    return out


def _emit(nc, xT_d, k_d, ident_d, o_d, tc):
    cfg = CFG
    f16 = mybir.dt.float16
    f32 = mybir.dt.float32
    with (
        tc.tile_pool(name="xwp", bufs=1) as xwp,
        tc.tile_pool(name="idp", bufs=1) as idp,
        tc.tile_pool(name="kerTp", bufs=3) as kerTp,
        tc.tile_pool(name="prp", bufs=PR_BUFS) as prp,
        tc.tile_pool(name="scp", bufs=PR_BUFS) as scp,
        tc.tile_pool(name="obp", bufs=1) as obp,
        tc.tile_pool(name="otp", bufs=3) as otp,
        tc.tile_pool(name="psp", bufs=3, space="PSUM") as psp,
    ):
        # Two small leading blocks shrink the startup ramp.
        if DMA_LEAN:
            blocks = [(0, 4), (4, 16), (16, 48), (48, 80), (80, 112),
                      (112, 128)]
            KTE = 32
        else:
            blocks = [(0, 4), (4, 16)] + [(b, b + IB)
                                          for b in range(16, BLK, IB)]
            KTE = IB

        def emit_kerT(b0, b1):
            t = kerTp.tile([BLK, KTE * K2P], f16, tag="kerT")
            t4 = t.rearrange("p (e k) -> p e k", e=b1 - b0)
            nc.sync.dma_start(out=t4[:, :, 0:K2P], in_=k_d[:, b0:b1, :])
            return t4

        preT = {}
        preT[blocks[0]] = emit_kerT(*blocks[0])

        # Sliding col-windows of transposed x:
        # xwE[p, c, v, r] = xpad[c, r,   p+v]   (even-i reads start at r=i)
        # xwO[p, c, v, r] = xpad[c, r+1, p+v]   (odd-i reads start at r=i-1)
        xwinE = xwp.tile([BLK, C * L * XSP], f16, tag="xwE")
        xwinO = xwp.tile([BLK, C * L * XSP], f16, tag="xwO")
        xwE = xwinE.rearrange("p (c v r) -> p c v r", c=C, v=L, r=XSP)
        xwO = xwinO.rearrange("p (c v r) -> p c v r", c=C, v=L, r=XSP)
        # dest[p, c, v, r] = xT[c, p+v, r (+1 odd)] -- the v dim overlaps the
        # partition dim (same stride), built as a manual AP.
        if WIN_MERGE:
            # host ships the window tiles verbatim: one flat contiguous
            # DMA per window (large linear descriptors), kerT prefetches
            # interleaved between them
            nc.sync.dma_start(out=xwinE[:, :], in_=xT_d[0])
            preT[blocks[1]] = emit_kerT(*blocks[1])
            nc.sync.dma_start(out=xwinO[:, :], in_=xT_d[1])
            preT[blocks[2]] = emit_kerT(*blocks[2])
        else:
            echunks = ((0, 48), (48, 96), (96, XSP))
            ochunks = ((0, 47), (47, 95), (95, XS + 1))
            for n, (r0, r1) in enumerate(echunks):
                for c in range(C):
                    src = AP(xT_d.tensor, c * XSP * XSP + r0,
                             [[XSP, BLK], [XSP, L], [1, r1 - r0]])
                    nc.sync.dma_start(out=xwE[:, c, :, r0:r1], in_=src)
                o0, o1 = ochunks[n]
                if XWO_DMA:
                    for c in range(C):
                        src = AP(xT_d.tensor, c * XSP * XSP + o0 + 1,
                                 [[XSP, BLK], [XSP, L], [1, o1 - o0]])
                        nc.sync.dma_start(out=xwO[:, c, :, o0:o1], in_=src)
                else:
                    nc.gpsimd.tensor_copy(xwO[:, :, :, o0:o1],
                                          xwE[:, :, :, o0 + 1:o1 + 1])
                if n == 0:
                    preT[blocks[1]] = emit_kerT(*blocks[1])
                elif n == 1:
                    preT[blocks[2]] = emit_kerT(*blocks[2])

        ident = idp.tile([BLK, BLK], f32)
        nc.sync.dma_start(out=ident[:, :], in_=ident_d)

        out_sb = obp.tile([BLK, C * BLK], f32)
        ob3 = out_sb.rearrange("p (c i) -> p c i", c=C)
        if cfg in ("noreduce", "dmaonly"):
            nc.vector.memset(out_sb[:, :], 0.0)

        # engine schedule for per-(c,i) reduces on non-TR rows
        sched = _split_engines(C * BLK, SPLIT_W)

        def emit_row_stt(i, xsl, k3):
            """TT covers ACT channels only; DVE channels fused via STT."""
            if ACT2_ALT:
                act2 = (i % 2) == 0
            else:
                act2 = (i % 4) < ACT2_OF4
            act_cs = (0, 2) if act2 else (1,)
            dve_cs = (1,) if act2 else (0, 2)
            prod = prp.tile([BLK, C * NT], f16, tag="prod")
            pr4 = prod.rearrange("p (c v u) -> p c v u", c=C, v=L)
            pr2 = prod.rearrange("p (c t) -> p c t", c=C)
            if act2:
                kb = k3.unsqueeze(1).broadcast_to([BLK, 2, L, VS])
                nc.vector.tensor_tensor(
                    out=pr4[:, 0:3:2], in0=xsl[:, 0:3:2], in1=kb,
                    op=mybir.AluOpType.mult)
            else:
                nc.vector.tensor_tensor(
                    out=pr4[:, 1], in0=xsl[:, 1], in1=k3,
                    op=mybir.AluOpType.mult)
            for c in dve_cs:
                scr = scp.tile([BLK, NT], f16, tag="scr")
                scr3 = scr.rearrange("p (v u) -> p v u", v=L)
                nc.vector.scalar_tensor_tensor(
                    out=scr3, in0=xsl[:, c], scalar=1.0, in1=k3,
                    op0=mybir.AluOpType.mult, op1=mybir.AluOpType.mult,
                    accum_out=ob3[:, c, i:i + 1])
            for c in act_cs:
                scr = scp.tile([BLK, NT], f16, tag="scr")
                nc.scalar.activation(
                    out=scr[:, :], in_=pr2[:, c, :],
                    func=mybir.ActivationFunctionType.Copy,
                    scale=1.0, accum_out=ob3[:, c, i:i + 1])

        for (b0, b1) in blocks:
            kerT4 = preT.get((b0, b1)) or emit_kerT(b0, b1)
            for ii in range(b1 - b0):
                i = b0 + ii
                if cfg == "dmaonly":
                    continue
                if i % 2 == 0:
                    xsl = xwE[:, :, :, i:i + VS]
                else:
                    xsl = xwO[:, :, :, i - 1:i - 1 + VS]
                k3 = kerT4[:, ii, 0:NT].rearrange("p (v u) -> p v u", v=L)
                if STT_MODE and cfg == "v8" and i % 8 not in POOL_ROWS \
                        and i % 8 not in TR_ROWS:
                    emit_row_stt(i, xsl, k3)
                    continue
                kb = k3.unsqueeze(1).broadcast_to([BLK, C, L, VS])
                prod = prp.tile([BLK, C * NT], f16, tag="prod")
                pr4 = prod.rearrange("p (c v u) -> p c v u", c=C, v=L)
                # ONE fused multiply for all channels: 2x fp16 TT.
                if i % 8 in POOL_ROWS:
                    nc.gpsimd.tensor_tensor(
                        out=pr4, in0=xsl, in1=kb, op=mybir.AluOpType.mult)
                else:
                    nc.vector.tensor_tensor(
                        out=pr4, in0=xsl, in1=kb, op=mybir.AluOpType.mult)
                if cfg == "noreduce":
                    continue
                pr2 = prod.rearrange("p (c t) -> p c t", c=C)
                if i % 8 in TR_ROWS:
                    # one grouped reduce for all 3 channels on DVE
                    nc.vector.tensor_reduce(
                        out=ob3[:, :, i:i + 1],
                        in_=pr2,
                        axis=mybir.AxisListType.X,
                        op=mybir.AluOpType.add)
                    continue
                for c in range(C):
                    eng = sched[i * C + c]
                    scr = scp.tile([BLK, NT], f16, tag="scr")
                    if eng == 0:
                        nc.vector.tensor_scalar(
                            out=scr[:, :],
                            in0=pr2[:, c, :],
                            scalar1=1.0,
                            scalar2=None,
                            op0=mybir.AluOpType.mult,
                            op1=mybir.AluOpType.add,
                            accum_out=ob3[:, c, i:i + 1],
                        )
                    else:
                        nc.scalar.activation(
                            out=scr[:, :],
                            in_=pr2[:, c, :],
                            func=mybir.ActivationFunctionType.Copy,
                            scale=1.0,
                            accum_out=ob3[:, c, i:i + 1],
                        )

        # Transpose [j, (c, i)] -> [i, (c, j)] via PE; scale by 1/361 in the
        # PSUM->SBUF copy; clean DMAs out.
        if DMA_LEAN:
            otb = otp.tile([BLK, C * BLK], f32, tag="otb")
            ot3 = otb.rearrange("p (c j) -> p c j", c=C)
            for c in range(C):
                ps = psp.tile([BLK, BLK], f32, tag="ps")
                nc.tensor.transpose(ps[:, :], ob3[:, c, :], ident[:, :])
                nc.scalar.activation(out=ot3[:, c, :], in_=ps[:, :],
                                     func=mybir.ActivationFunctionType.Copy,
                                     scale=1.0 / K2)
            dst = AP(o_d.tensor, 0, [[BLK, BLK], [BLK * BLK, C], [1, BLK]])
            nc.sync.dma_start(out=dst, in_=otb[:, :])
        else:
            for c in range(C):
                ps = psp.tile([BLK, BLK], f32, tag="ps")
                nc.tensor.transpose(ps[:, :], ob3[:, c, :], ident[:, :])
                ot = otp.tile([BLK, BLK], f32, tag="ot")
                nc.scalar.activation(out=ot[:, :], in_=ps[:, :],
                                     func=mybir.ActivationFunctionType.Copy,
                                     scale=1.0 / K2)
                nc.sync.dma_start(out=o_d[c], in_=ot[:, :])


def build_program(repeat=1):
    key = ("nc", repeat, CFG, XWO_DMA, POOL_ROWS, TR_ROWS, SPLIT_W,
           STT_MODE, ACT2_OF4, ACT2_ALT, PR_BUFS, WIN_MERGE, DMA_LEAN)
    if key in _CACHE:
        return _CACHE[key]
    nc = bacc.Bacc(
        "TRN2",
        target_bir_lowering=False,
        debug=False,
        enable_asserts=True,
        num_devices=8,
    )
    f16 = mybir.dt.float16
    f32 = mybir.dt.float32
    if WIN_MERGE:
        xT_d = nc.dram_tensor("xT", [2, BLK, C * L * XSP], f16,
                              kind="ExternalInput").ap()
    else:
        xT_d = nc.dram_tensor("xT", [C, XSP, XSP], f16,
                              kind="ExternalInput").ap()
    k_d = nc.dram_tensor("ker", [BLK, BLK, K2P], f16,
                         kind="ExternalInput").ap()
    ident_d = nc.dram_tensor("ident", [BLK, BLK], f32,
                             kind="ExternalInput").ap()
    o_d = nc.dram_tensor("out", [C, BLK, BLK], f32, kind="ExternalOutput").ap()
    with tile.TileContext(nc) as tc:
        if repeat > 1:
            with tc.For_i(0, repeat):
                _emit(nc, xT_d, k_d, ident_d, o_d, tc)
        else:
            _emit(nc, xT_d, k_d, ident_d, o_d, tc)
    nc.compile()
    _CACHE[key] = nc
    return nc


def shard_inputs(input, kernel):
    xpad = np.pad(input, ((0, 0), (0, 0), (PAD, PAD), (PAD, PAD)),
                  mode="reflect")
    ident = np.eye(BLK, dtype=np.float32)
    in_maps = []
    for core in range(8):
        b, hh, wh = core >> 2, (core >> 1) & 1, core & 1
        xs = xpad[b, :, hh * BLK:hh * BLK + XS, wh * BLK:wh * BLK + XS]
        xT = np.zeros((C, XSP, XSP), dtype=np.float16)
        xT[:, :XS, :XS] = xs.transpose(0, 2, 1).astype(np.float16)
        if WIN_MERGE:
            he = np.empty((BLK, C, L, XSP), dtype=np.float16)
            for v in range(L):
                he[:, :, v, :] = xT[:, v:v + BLK, :].transpose(1, 0, 2)
            ho = np.zeros_like(he)
            ho[..., :XSP - 1] = he[..., 1:]
            xT = np.stack([he.reshape(BLK, C * L * XSP),
                           ho.reshape(BLK, C * L * XSP)])
        ks = kernel[b, :, hh * BLK:(hh + 1) * BLK, wh * BLK:(wh + 1) * BLK]
        src = ks.astype(np.float16).reshape(L, L, BLK, BLK)  # [u, v, i, j]
        # dest slot k'' = v*20 + u  <-  source tap (u, v); u=19 slot zero
        ksp = np.zeros((BLK, BLK, K2P), dtype=np.float16)
        tmp = np.zeros((BLK, BLK, L, VS), dtype=np.float16)
        tmp[:, :, :, :L] = src.transpose(3, 2, 1, 0)  # [j, i, v, u]
        ksp[:, :, :L * VS] = tmp.reshape(BLK, BLK, L * VS)
        in_maps.append({"xT": xT, "ker": ksp, "ident": ident})
    return in_maps


def gather_outputs(results):
    out = np.empty((B, C, H, W), dtype=np.float32)
    for core in range(8):
        b, hh, wh = core >> 2, (core >> 1) & 1, core & 1
        out[b, :, hh * BLK:(hh + 1) * BLK, wh * BLK:(wh + 1) * BLK] = \
            results[core]["out"]
    return out


def kernel(input, kernel):
    global LAST_EXEC_NS
    nc = build_program()
    in_maps = shard_inputs(np.asarray(input, dtype=np.float32),
                           np.asarray(kernel, dtype=np.float32))
    res = bass_utils.run_bass_kernel_spmd(
        nc, in_maps, core_ids=list(range(8)))
    LAST_EXEC_NS = res.exec_time_ns
    return gather_outputs(res.results)
